# revision 5
# baseline (speedup 1.0000x reference)
"""AttentionMemoryFusion kernel for 8 TRN2 NeuronCores (Bass/Tile, SPMD).

Math (refactored from the reference):
  q      = cf @ Wq.T + bq                      [B, HD]
  keys   = mem @ Wk.T          (bk drops out of softmax — per-row const)
  s      = (q @ keys.T) / sqrt(HD)             [B, M]
  w      = exp(s)   (no max subtraction needed: |s| <~ 1)
  ctx    = (w @ mem) / rowsum(w)               [B, D]
  fused  = cf @ Wo1.T + ctx @ (Wo2 @ Wv).T + (Wo2 @ bv + bo)   [B, D]
  mem_new= mem with rows 0..B-1 replaced by cf

Sharding: memory rows split 8 ways (8192 rows/core). Each core computes
flash-style partial ctx^T + partial softmax denominator over its shard,
a ReduceScatter(add) over a B-major [1024, 513] buffer hands core c the
fully-reduced ctx rows for B-block c (plus denominator column), and each
core computes the output projection for its B-block. mem_new is written
by streaming the shard through SBUF (head rows come from a per-core
"head" input so the scatter of cf into rows 0..1023 happens on device).
"""

import sys

if "/opt/trn_rl_repo" not in sys.path:
    sys.path.insert(0, "/opt/trn_rl_repo")

import numpy as np

import concourse.bass as bass
import concourse.tile as tile
from concourse import bacc, mybir
from concourse import bass_utils
from concourse.masks import make_identity

B, D, M, HD = 1024, 512, 65536, 64
NC = 8
MS = M // NC            # 8192 memory rows per core
NCH = MS // 128         # 64 m-chunks of 128 rows
DC = D // 128           # 4 D-chunks
HEAD = B                # rows replaced at ptr=0
F32 = mybir.dt.float32
BF16 = mybir.dt.bfloat16
SCALE = 1.0 / float(np.sqrt(HD, dtype=np.float32))

_CACHE = {}


def _build_module():
    nc = bacc.Bacc("TRN2", target_bir_lowering=False, debug=False, num_devices=NC)

    mem_in = nc.dram_tensor("mem", [MS, D], F32, kind="ExternalInput").ap()
    head_in = nc.dram_tensor("head", [HEAD, D], F32, kind="ExternalInput").ap()
    cfT_in = nc.dram_tensor("cfT", [D, B], F32, kind="ExternalInput").ap()
    cfTs_in = nc.dram_tensor("cfTs", [D, 128], F32, kind="ExternalInput").ap()
    wqT_in = nc.dram_tensor("wqT", [D, HD], F32, kind="ExternalInput").ap()
    wkT_in = nc.dram_tensor("wkT", [D, HD], F32, kind="ExternalInput").ap()
    wovT_in = nc.dram_tensor("wovT", [D, D], F32, kind="ExternalInput").ap()
    wo1T_in = nc.dram_tensor("wo1T", [D, D], F32, kind="ExternalInput").ap()
    bq_in = nc.dram_tensor("bq", [HD, 1], F32, kind="ExternalInput").ap()
    bias2_in = nc.dram_tensor("bias2", [D, 1], F32, kind="ExternalInput").ap()

    mem_out = nc.dram_tensor("mem_out", [MS, D], F32, kind="ExternalOutput").ap()
    fusedT_out = nc.dram_tensor("fusedT_out", [D, 128], F32, kind="ExternalOutput").ap()

    with tile.TileContext(nc) as tc:
        with (
            tc.tile_pool(name="res", bufs=1) as res,
            tc.tile_pool(name="dram", bufs=1, space="DRAM") as dram,
        ):
            # ---- resident SBUF tensors ----
            mem_bf = res.tile([128, NCH * D], BF16)      # natural mem, bf16
            keysT_bf = res.tile([HD, MS], BF16)          # keys^T
            qT_bf = res.tile([HD, B], BF16)
            cfT_bf = res.tile([128, DC * B], BF16)
            cfTs_bf = res.tile([128, D], BF16)
            wkT_bf = res.tile([128, DC * HD], BF16)
            wovT_bf = res.tile([128, DC * D], BF16)
            wo1T_bf = res.tile([128, DC * D], BF16)
            ident_bf = res.tile([128, 128], BF16)
            ident_f = res.tile([128, 128], F32)
            ones_bf = res.tile([128, 1], BF16)
            bq_sb = res.tile([HD, 1], F32)
            bias2_sb = res.tile([128, DC], F32)
            den_sb = res.tile([128, B // 2], F32)

            rs_in = dram.tile([B, D + 1], F32)
            rs_out = dram.tile([B // NC, D + 1], F32)

            make_identity(nc, ident_bf[:])
            make_identity(nc, ident_f[:])
            nc.gpsimd.memset(ones_bf[:], 1.0)
            nc.gpsimd.memset(den_sb[:], 0.0)
            nc.sync.dma_start(bq_sb[:], bq_in[:])
            for d in range(DC):
                nc.sync.dma_start(bias2_sb[:, d : d + 1], bias2_in[128 * d : 128 * (d + 1), :])

            # ---- load + cast small weights ----
            with tc.tile_pool(name="stage", bufs=3) as stage:
                for d in range(DC):
                    st = stage.tile([128, B], F32, tag="stage")
                    nc.sync.dma_start(st[:], cfT_in[128 * d : 128 * (d + 1), :])
                    nc.vector.tensor_copy(cfT_bf[:, B * d : B * (d + 1)], st[:])
                st = stage.tile([128, D], F32, tag="stage")
                for d in range(DC):
                    nc.sync.dma_start(
                        st[:, 128 * d : 128 * (d + 1)], cfTs_in[128 * d : 128 * (d + 1), :]
                    )
                nc.vector.tensor_copy(cfTs_bf[:], st[:])
                for d in range(DC):
                    st = stage.tile([128, D], F32, tag="stage")
                    nc.sync.dma_start(st[:, 0:HD], wkT_in[128 * d : 128 * (d + 1), :])
                    nc.vector.tensor_copy(wkT_bf[:, HD * d : HD * (d + 1)], st[:, 0:HD])
                for d in range(DC):
                    st = stage.tile([128, D], F32, tag="stage")
                    nc.sync.dma_start(st[:], wovT_in[128 * d : 128 * (d + 1), :])
                    nc.vector.tensor_copy(wovT_bf[:, D * d : D * (d + 1)], st[:])
                for d in range(DC):
                    st = stage.tile([128, D], F32, tag="stage")
                    nc.sync.dma_start(st[:], wo1T_in[128 * d : 128 * (d + 1), :])
                    nc.vector.tensor_copy(wo1T_bf[:, D * d : D * (d + 1)], st[:])

            # wq needs its own resident tile (used in qT matmuls)
            wqT_bf = res.tile([128, DC * HD], BF16)
            with tc.tile_pool(name="stage2", bufs=2) as stage2:
                for d in range(DC):
                    st = stage2.tile([128, HD], F32, tag="s2")
                    nc.sync.dma_start(st[:], wqT_in[128 * d : 128 * (d + 1), :])
                    nc.vector.tensor_copy(wqT_bf[:, HD * d : HD * (d + 1)], st[:])

            # ---- head rows of mem_new (scatter target) ----
            nc.sync.dma_start(mem_out[0:HEAD, :], head_in[:])

            # ---- phase 1: stream mem shard; writeback; cast; keys^T ----
            with (
                tc.tile_pool(name="ld", bufs=4) as ld,
                tc.tile_pool(name="memT", bufs=3) as memTp,
                tc.tile_pool(name="tp_a", bufs=2, space="PSUM") as tp_a,
                tc.tile_pool(name="kp", bufs=2, space="PSUM") as kp,
            ):
                # q^T = Wq @ cf^T + bq  (two N=512 halves)
                for h in range(2):
                    qp = kp.tile([HD, 512], F32, tag="kp")
                    for d in range(DC):
                        nc.tensor.matmul(
                            qp[:],
                            wqT_bf[:, HD * d : HD * (d + 1)],
                            cfT_bf[:, B * d + 512 * h : B * d + 512 * (h + 1)],
                            start=(d == 0),
                            stop=(d == DC - 1),
                        )
                    nc.vector.tensor_scalar_add(qT_bf[:, 512 * h : 512 * (h + 1)], qp[:], bq_sb[:])

                for i in range(NCH):
                    mc = ld.tile([128, D], F32, tag="ld")
                    nc.sync.dma_start(mc[:], mem_in[128 * i : 128 * (i + 1), :])
                    if i >= HEAD // 128:
                        nc.sync.dma_start(mem_out[128 * i : 128 * (i + 1), :], mc[:])
                    nc.gpsimd.tensor_copy(mem_bf[:, D * i : D * (i + 1)], mc[:])
                    mt = memTp.tile([128, D], BF16, tag="memT")
                    for d in range(DC):
                        tp = tp_a.tile([128, 128], BF16, tag="tp_a")
                        nc.tensor.transpose(
                            tp[:], mem_bf[:, D * i + 128 * d : D * i + 128 * (d + 1)], ident_bf[:]
                        )
                        nc.scalar.copy(mt[:, 128 * d : 128 * (d + 1)], tp[:])
                    kt = kp.tile([HD, 128], F32, tag="kp")
                    for d in range(DC):
                        nc.tensor.matmul(
                            kt[:],
                            wkT_bf[:, HD * d : HD * (d + 1)],
                            mt[:, 128 * d : 128 * (d + 1)],
                            start=(d == 0),
                            stop=(d == DC - 1),
                        )
                    nc.vector.tensor_copy(keysT_bf[:, 128 * i : 128 * (i + 1)], kt[:])

            # ---- phase 2+3: per B-half flash pass + B-major bounce ----
            with (
                tc.tile_pool(name="ctxp", bufs=1, space="PSUM") as ctxp,
                tc.tile_pool(name="scp", bufs=2, space="PSUM") as scp,
                tc.tile_pool(name="denp", bufs=1, space="PSUM") as denp,
                tc.tile_pool(name="tp_b", bufs=1, space="PSUM") as tp_b,
                tc.tile_pool(name="wT", bufs=4) as wTp,
                tc.tile_pool(name="wsum", bufs=2) as wsump,
                tc.tile_pool(name="ctxsb", bufs=1) as ctxsbp,
                tc.tile_pool(name="nat", bufs=2) as natp,
            ):
                for h in range(2):
                    ctx_ps = ctxp.tile([128, DC * 512], F32, tag="ctxp")
                    wsum = wsump.tile([128, 512], F32, tag="wsum")
                    for i in range(NCH):
                        sc = scp.tile([128, 512], F32, tag="scp")
                        nc.tensor.matmul(
                            sc[:],
                            keysT_bf[:, 128 * i : 128 * (i + 1)],
                            qT_bf[:, 512 * h : 512 * (h + 1)],
                            start=True,
                            stop=True,
                        )
                        wt = wTp.tile([128, 512], BF16, tag="wT")
                        nc.scalar.activation(
                            wt[:], sc[:], mybir.ActivationFunctionType.Exp, scale=SCALE
                        )
                        if i == 0:
                            nc.vector.tensor_copy(wsum[:], wt[:])
                        else:
                            nc.vector.tensor_add(wsum[:], wsum[:], wt[:])
                        for d in range(DC):
                            nc.tensor.matmul(
                                ctx_ps[:, 512 * d : 512 * (d + 1)],
                                mem_bf[:, D * i + 128 * d : D * i + 128 * (d + 1)],
                                wt[:],
                                start=(i == 0),
                                stop=(i == NCH - 1),
                                skip_group_check=True,
                            )
                    # denominator: ones^T @ wsum  (partition reduction)
                    wsum_bf = wsump.tile([128, 512], BF16, tag="wsumbf")
                    nc.vector.tensor_copy(wsum_bf[:], wsum[:])
                    dp = denp.tile([1, 512], F32, tag="denp")
                    nc.tensor.matmul(dp[:], ones_bf[:], wsum_bf[:], start=True, stop=True)
                    nc.scalar.copy(den_sb[0:1, :], dp[:])

                    ctx_sb = ctxsbp.tile([128, DC * 512], F32, tag="ctxsb")
                    nc.scalar.copy(ctx_sb[:], ctx_ps[:])

                    # transpose ctx^T -> B-major, append denominator column
                    for b in range(4):
                        nat = natp.tile([128, D + 1], F32, tag="nat")
                        for d in range(DC):
                            tp = tp_b.tile([128, 128], F32, tag="tp_b")
                            nc.tensor.transpose(
                                tp[:],
                                ctx_sb[:, 512 * d + 128 * b : 512 * d + 128 * (b + 1)],
                                ident_f[:],
                            )
                            nc.scalar.copy(nat[:, 128 * d : 128 * (d + 1)], tp[:])
                        tp = tp_b.tile([128, 128], F32, tag="tp_b")
                        nc.tensor.transpose(tp[:], den_sb[:, 128 * b : 128 * (b + 1)], ident_f[:])
                        nc.vector.tensor_copy(nat[:, D : D + 1], tp[:, 0:1])
                        nc.sync.dma_start(
                            rs_in[128 * (4 * h + b) : 128 * (4 * h + b + 1), :], nat[:]
                        )

            # ---- phase 4: cross-core reduce ----
            nc.gpsimd.collective_compute(
                "ReduceScatter",
                mybir.AluOpType.add,
                replica_groups=[list(range(NC))],
                ins=[rs_in[:].opt()],
                outs=[rs_out[:].opt()],
            )

            # ---- phase 5: epilogue for this core's B-block ----
            with (
                tc.tile_pool(name="ep", bufs=1) as ep,
                tc.tile_pool(name="tp_c", bufs=2, space="PSUM") as tp_c,
                tc.tile_pool(name="fp", bufs=1, space="PSUM") as fp,
            ):
                ctxn_in = ep.tile([128, D + 1], F32, tag="ep_in")
                nc.sync.dma_start(ctxn_in[:], rs_out[:])
                recip = ep.tile([128, 1], F32, tag="ep_r")
                nc.vector.reciprocal(recip[:], ctxn_in[:, D : D + 1])
                ctxn_bf = ep.tile([128, D], BF16, tag="ep_nbf")
                nc.vector.tensor_scalar_mul(ctxn_bf[:], ctxn_in[:, 0:D], recip[:])
                ctxnT_bf = ep.tile([128, D], BF16, tag="ep_tbf")
                for d in range(DC):
                    tp = tp_c.tile([128, 128], BF16, tag="tp_c")
                    nc.tensor.transpose(
                        tp[:], ctxn_bf[:, 128 * d : 128 * (d + 1)], ident_bf[:]
                    )
                    nc.scalar.copy(ctxnT_bf[:, 128 * d : 128 * (d + 1)], tp[:])
                fused_sb = ep.tile([128, D], F32, tag="ep_out")
                for do in range(DC):
                    fps = fp.tile([128, 128], F32, tag="fp")
                    for k in range(DC):
                        nc.tensor.matmul(
                            fps[:],
                            wovT_bf[:, D * k + 128 * do : D * k + 128 * (do + 1)],
                            ctxnT_bf[:, 128 * k : 128 * (k + 1)],
                            start=(k == 0),
                            stop=False,
                            skip_group_check=True,
                        )
                    for k in range(DC):
                        nc.tensor.matmul(
                            fps[:],
                            wo1T_bf[:, D * k + 128 * do : D * k + 128 * (do + 1)],
                            cfTs_bf[:, 128 * k : 128 * (k + 1)],
                            start=False,
                            stop=(k == DC - 1),
                            skip_group_check=True,
                        )
                    nc.vector.tensor_scalar_add(
                        fused_sb[:, 128 * do : 128 * (do + 1)], fps[:], bias2_sb[:, do : do + 1]
                    )
                    nc.sync.dma_start(
                        fusedT_out[128 * do : 128 * (do + 1), :],
                        fused_sb[:, 128 * do : 128 * (do + 1)],
                    )

    nc.compile()
    return nc


def _get_module():
    if "nc" not in _CACHE:
        _CACHE["nc"] = _build_module()
    return _CACHE["nc"]


def _prepare_in_maps(current_features, memory, Wq, bq, Wk, bk, Wv, bv, Wo, bo):
    cf = np.asarray(current_features, np.float32)
    memory = np.asarray(memory, np.float32)
    Wq, bq = np.asarray(Wq, np.float32), np.asarray(bq, np.float32)
    Wk = np.asarray(Wk, np.float32)
    Wv, bv = np.asarray(Wv, np.float32), np.asarray(bv, np.float32)
    Wo, bo = np.asarray(Wo, np.float32), np.asarray(bo, np.float32)

    Wo1, Wo2 = Wo[:, :D], Wo[:, D:]
    cfT = np.ascontiguousarray(cf.T)
    shared = {
        "cfT": cfT,
        "wqT": np.ascontiguousarray(Wq.T),
        "wkT": np.ascontiguousarray(Wk.T),
        "wovT": np.ascontiguousarray((Wo2 @ Wv).T),
        "wo1T": np.ascontiguousarray(Wo1.T),
        "bq": np.ascontiguousarray(bq.reshape(HD, 1)),
        "bias2": np.ascontiguousarray((Wo2 @ bv + bo).reshape(D, 1)),
    }
    in_maps = []
    for c in range(NC):
        m = dict(shared)
        m["mem"] = np.ascontiguousarray(memory[MS * c : MS * (c + 1)])
        m["head"] = cf if c == 0 else np.ascontiguousarray(memory[MS * c : MS * c + HEAD])
        m["cfTs"] = np.ascontiguousarray(cfT[:, 128 * c : 128 * (c + 1)])
        in_maps.append(m)
    return in_maps


def _assemble(res):
    fusedT = np.concatenate([res.results[c]["fusedT_out"] for c in range(NC)], axis=1)
    mem_new = np.concatenate([res.results[c]["mem_out"] for c in range(NC)], axis=0)
    return np.ascontiguousarray(fusedT.T), mem_new


def kernel(**inputs):
    in_maps = _prepare_in_maps(**inputs)
    nc = _get_module()
    res = bass_utils.run_bass_kernel_spmd(nc, in_maps, core_ids=list(range(NC)))
    return _assemble(res)


def run_traced(**inputs):
    in_maps = _prepare_in_maps(**inputs)
    nc = _get_module()
    res = bass_utils.run_bass_kernel_spmd(
        nc, in_maps, core_ids=list(range(NC)), trace=True
    )
    res.outputs = _assemble(res)
    return res
